# revision 10
# baseline (speedup 1.0000x reference)
"""Confusion-matrix (joint histogram) kernel for Trainium2.

Math: out[b, i, j] = #{pixels p in batch b : yp[b,p] == i and y[b,p] == j}
for i, j in [0, 21). Inputs yp, y are [8, 2048, 2048] int32, values in [0, 21).

Strategy (per NeuronCore, core c processes batch c):
  - THERMOMETER encoding: planes[level i] = (x >= i) instead of one-hots.
    TensorE then accumulates TH[i, j] = #{yp >= i and y >= j}; the host
    recovers counts via an exact 2-D finite difference (integer math in
    float64). Thermometer levels are cheaper to generate than one-hots:
      level 0  = all-ones const plane (memset once per buffer, never redone)
      level 1  = Sign(x) on ScalarE (1 op)
      level 19 = Sign(Relu(x - 18)) on ScalarE (2 ops)
      level 20 = Relu(2x - 39) on ScalarE (1 op)
      levels 2..18 = tensor_scalar(is_ge) on VectorE (17 ops, 4x perf mode)
  - layout: planes[p, blk*126 + i*6 + g], 6 pixel-column groups per matmul
    ([128, 126] x [128, 126]) accumulated into one PSUM [126, 126] f32 tile
    (exact integer counts < 2^24),
  - tail padding uses value 0 -> pads land in TH[0,0] only; the host
    subtracts the deterministic pad count from bin [0, 0].

GpSimd is left COMPLETELY idle: its SBUF port is shared with VectorE and
any sustained GpSimd traffic destroys the DVE 4x (two-port) perf mode
(measured 8x DVE slowdown + 16us/op GpSimd is_equal).
"""

import numpy as np

C = 21                  # classes / thermometer levels
G = 6                   # pixel-column groups per matmul (G*C = 126 <= 128)
M = G * C               # 126
P = 128                 # partitions
FP = 1008               # plane-chunk columns per tensor (divisible by 6)
MASK_DT = "bf16"

_CACHE = {}


def _build(
    n_free,
    work_cols=None,
    repeat=1,
    skip_mm=False,
    n_cls=C,
    mask_dt=MASK_DT,
):
    import concourse.bacc as bacc
    import concourse.mybir as mybir
    import concourse.tile as tile
    from contextlib import nullcontext

    if work_cols is None:
        work_cols = n_free

    nc = bacc.Bacc(
        "TRN2",
        target_bir_lowering=False,
        debug=False,
        enable_asserts=False,
        num_devices=8,
    )
    yp = nc.dram_tensor("yp", [P, n_free], mybir.dt.int32, kind="ExternalInput").ap()
    y = nc.dram_tensor("y", [P, n_free], mybir.dt.int32, kind="ExternalInput").ap()
    out = nc.dram_tensor("out", [M, M], mybir.dt.float32, kind="ExternalOutput").ap()

    # Graduated chunk widths: small first chunks cut the pipeline-fill
    # latency (PE can start after ~250 cols instead of a full FP chunk);
    # the remainder becomes a small padded tail chunk.
    widths = []
    remaining = work_cols
    for wsmall in (252, 504):
        if remaining >= wsmall + FP:
            widths.append(wsmall)
            remaining -= wsmall
    widths += [FP] * (remaining // FP)
    tail_cols = remaining - (remaining // FP) * FP   # < FP
    tail_pad = -tail_cols % G
    tail_w = tail_cols + tail_pad
    total_mms = sum(w // G for w in widths) + (tail_w // G)

    mdt = {"bf16": mybir.dt.bfloat16, "fp8": mybir.dt.float8e4}[mask_dt]
    bf16 = mybir.dt.bfloat16
    f32 = mybir.dt.float32
    i32 = mybir.dt.int32
    Copy = mybir.ActivationFunctionType.Copy
    Relu = mybir.ActivationFunctionType.Relu
    Sign = mybir.ActivationFunctionType.Sign
    dve_lvls = list(range(2, n_cls - 2))             # 2..18 on VectorE
    NBLK = 2 * FP // G                               # blocks at full width

    with tile.TileContext(nc) as tc:
        with (
            tc.tile_pool(name="psum", bufs=1, space="PSUM") as psum_pool,
            tc.tile_pool(name="cat", bufs=2) as cat_pool,
            tc.tile_pool(name="sq", bufs=1) as sq_pool,
            tc.tile_pool(name="singles", bufs=1) as singles,
        ):
            acc = psum_pool.tile([M, M], f32)
            bias_m18 = singles.tile([P, 1], f32, tag="bias18")
            nc.vector.memset(bias_m18[:], -float(n_cls - 3))
            bias_ramp = singles.tile([P, 1], f32, tag="biasr")
            nc.vector.memset(bias_ramp[:], -(2.0 * (n_cls - 1) - 1.0))

            # Two manually ping-ponged plane buffers; their level-0 slice is
            # an all-ones constant written once and never touched again.
            plane_bufs = []
            for pb in range(2):
                pl = singles.tile([P, C * 2 * FP], mdt, tag=f"planes{pb}")
                pl3f = pl[:].rearrange("p (b f) -> p b f", f=M)
                nc.vector.memset(pl3f[:, :, 0:G], 1.0)
                plane_bufs.append(pl)

            mm = 0
            chunk_idx = 0
            rep_ctx = tc.For_i(0, repeat, 1) if repeat > 1 else nullcontext()

            with rep_ctx:

                def do_plane_chunk(cat32, w):
                    """cat32: [128, 2*w] int32 = [yp vals | y vals], w % 6 == 0.

                    planes[p, blk*126 + i*6 + g] = (vals[p, blk*6+g] >= i),
                    blk in [0, 2*w/6). A-side = blks [0, w/6), B-side = rest.
                    Each matmul reads a contiguous [128, 126] slice.
                    """
                    nonlocal mm, chunk_idx
                    nblk = 2 * w // G
                    cat16 = cat_pool.tile([P, 2 * FP], bf16, tag="cat16")
                    c16 = cat16[:, : 2 * w]
                    nc.scalar.activation(c16[:], cat32[:], Copy)
                    planes = plane_bufs[chunk_idx % 2]
                    chunk_idx += 1
                    pl3 = planes[:, : nblk * M].rearrange("p (b f) -> p b f", f=M)
                    cat3 = c16[:].rearrange("p (b f) -> p b f", f=G)
                    cat3_32 = cat32[:].rearrange("p (b f) -> p b f", f=G)
                    for i in dve_lvls:
                        nc.vector.tensor_scalar(
                            pl3[:, :, i * G : (i + 1) * G],
                            cat3[:],
                            float(i),
                            None,
                            mybir.AluOpType.is_ge,
                        )
                    # level 1: Sign(x) = (x >= 1) for x in {0..20}
                    nc.scalar.activation(
                        pl3[:, :, 1 * G : 2 * G], cat3_32[:], Sign, bias=0.0
                    )
                    # level 19: t = relu(x - 18) in {0,1,2}; Sign(t) = (x >= 19)
                    i = n_cls - 2
                    tsq = sq_pool.tile([P, 2 * FP], bf16, tag="tsq")
                    t3 = tsq[:, : 2 * w].rearrange("p (b f) -> p b f", f=G)
                    nc.scalar.activation(
                        t3[:], cat3_32[:], Relu, bias=bias_m18[:]
                    )
                    nc.scalar.activation(
                        pl3[:, :, i * G : (i + 1) * G], t3[:], Sign, bias=0.0
                    )
                    # level 20: relu(2x - 39) = (x >= 20) for x in {0..20}
                    i = n_cls - 1
                    nc.scalar.activation(
                        pl3[:, :, i * G : (i + 1) * G],
                        cat3_32[:],
                        Relu,
                        bias=bias_ramp[:],
                        scale=2.0,
                    )
                    half = (w // G) * M
                    for t in (range(0) if skip_mm else range(w // G)):
                        nc.tensor.matmul(
                            acc[:, :],
                            planes[:, t * M : (t + 1) * M],
                            planes[:, half + t * M : half + (t + 1) * M],
                            start=(mm == 0),
                            stop=(mm == total_mms - 1),
                        )
                        mm += 1

                off = 0
                for w in widths:
                    ct = cat_pool.tile([P, 2 * FP], i32, tag="cat32")
                    ctw = ct[:, : 2 * w]
                    nc.sync.dma_start(ctw[:, :w], yp[:, off : off + w])
                    nc.sync.dma_start(ctw[:, w:], y[:, off : off + w])
                    do_plane_chunk(ctw, w)
                    off += w

                if tail_cols:
                    ct = cat_pool.tile([P, 2 * FP], i32, tag="cat32")
                    ctw = ct[:, : 2 * tail_w]
                    if tail_pad:
                        # pad value 0: pad pixels land in TH[0,0] only; the
                        # host subtracts the deterministic pad count.
                        nc.vector.memset(ctw[:], 0)
                    nc.sync.dma_start(
                        ctw[:, :tail_cols], yp[:, off : off + tail_cols]
                    )
                    nc.sync.dma_start(
                        ctw[:, tail_w : tail_w + tail_cols],
                        y[:, off : off + tail_cols],
                    )
                    do_plane_chunk(ctw, tail_w)

            assert skip_mm or mm == total_mms
            res = singles.tile([M, M], f32)
            if skip_mm:
                nc.vector.memset(res[:], 0.0)
            else:
                nc.vector.tensor_copy(res[:], acc[:, :])
            nc.sync.dma_start(out, res[:])

    nc.compile()
    n_pad_px = tail_pad * P
    return nc, n_pad_px


def _get(n_free):
    if n_free not in _CACHE:
        _CACHE[n_free] = _build(n_free)
    return _CACHE[n_free]


def kernel(yp, y, res, n_classes, _trace=False):
    from concourse import bass_utils

    yp = np.ascontiguousarray(np.asarray(yp))
    y = np.ascontiguousarray(np.asarray(y))
    B = yp.shape[0]
    n_free = yp[0].size // P
    nc, n_pad_px = _get(n_free)
    in_maps = [
        {"yp": yp[b].reshape(P, n_free), "y": y[b].reshape(P, n_free)}
        for b in range(B)
    ]
    r = bass_utils.run_bass_kernel_spmd(
        nc, in_maps, core_ids=list(range(B)), trace=_trace
    )
    outs = []
    for b in range(B):
        Pm = r.results[b]["out"].astype(np.float64)
        TH = np.zeros((C, C), np.float64)
        for g in range(G):
            TH += Pm[g::G, g::G]
        TH[0, 0] -= n_pad_px
        # counts = 2-D finite difference of the cumulative (>=, >=) matrix
        THp = np.zeros((C + 1, C + 1), np.float64)
        THp[:C, :C] = TH
        Rb = THp[:C, :C] - THp[1:, :C] - THp[:C, 1:] + THp[1:, 1:]
        outs.append(Rb)
    res_np = np.stack(outs).astype(np.float32)
    if _trace:
        kernel._last_results = r
    return res_np
